# revision 2
# baseline (speedup 1.0000x reference)
"""Cached-attention kernel v3 for Trainium2 (8 NeuronCores, Bass/Tile).

Problem: B=4, L=2048 new tokens, S=2048 cached tokens, D=2048.
  Q = x @ Wq.T ; K = x @ Wk.T ; V = x @ Wv.T
  K_cal = concat(K, cache_k) ; V_cal = concat(V, cache_v)
  out = softmax(Q @ K_cal.T / sqrt(D)) @ V_cal

Sharding: 8 cores = (batch b in 0..3) x (key-half h in 0..1). Each core
runs ALL 2048 queries of its batch against its 2048 local keys (1024
cached + 1024 new). Softmax is flash-style un-normalized: the core
returns numerator^T [D, L] (bf16) and denominator [1, L] (f32); the
host sums the two halves and divides.

v3 vs v2: hand-drawn SBUF map (alloc_sbuf_tensor_at) so that
  - the x chunks of the kv-half are loaded once and reused by the V, K
    and Q(lh0) projections (v2 re-DMA'd 4MB and stalled ~16us);
  - wv weight chunks alias the (then-empty) qt block buffers;
  - pT exp tiles alias the consumed x buffers;
  - lh=1 x chunks are prefetched during the lh=0 attention phase;
  - next-rep x/wv loads only WAR against readers that finish early in
    the previous rep, so the For_i steady state has no cold start.
  Cached K^T/V resident loads go on the Pool queue, wv streams on the
  Activation queue; SP keeps only small weight tiles and outputs.

SBUF map (bytes per partition, base = 16512 after the bass reserve):
  [0,       65536)   kT   [128,16,2048] bf16 resident
  [65536,  131072)   v    [128,16,2048] bf16 resident
  [131072, 147456)   S1: qt block 0  | wv chunk A
  [147456, 163840)   S2: qt block 1  | wv chunk B
  [163840, 180224)   S3: x chunk 0   | pT (blocks 0,2)
  [180224, 196608)   S4: x chunk 1   | pT (blocks 1,3)
  [196608, 204800)   W:  wk/wq tile x2 (4KB each)
  [204800, 208896)   o staging x4 (1KB each)
  [208896, 212992)   d staging x2 (2KB each)
  [212992, ...)      ones
"""

import numpy as np
import ml_dtypes

import concourse.bass as bass
import concourse.tile as tile
from concourse import bacc, mybir
from concourse import bass2jax

F32 = mybir.dt.float32
BF16 = mybir.dt.bfloat16
BF16_NP = ml_dtypes.bfloat16

D = 2048          # model dim
L = 2048          # new tokens (queries)
HALF = 1024       # per-core share of new/cached keys; also query-half
NT = D // 128     # 16 tiles of 128 along D/E/S
SCALE = 1.0 / float(np.sqrt(D))
N_CORES = 8

_NC_CACHE = {}

BASE = 16512


def build_program(reps=1):
    key = ("nc", reps)
    if key in _NC_CACHE:
        return _NC_CACHE[key]
    nc = bacc.Bacc(None, target_bir_lowering=False, debug=False)
    xT = nc.dram_tensor("xT", [D, L], BF16, kind="ExternalInput")
    wq = nc.dram_tensor("wq", [128, NT, NT, 128], BF16, kind="ExternalInput")
    wk = nc.dram_tensor("wk", [128, NT, NT, 128], BF16, kind="ExternalInput")
    wvT = nc.dram_tensor("wvT", [D, D], BF16, kind="ExternalInput")
    kcT = nc.dram_tensor("kcT", [D, HALF], BF16, kind="ExternalInput")
    vc = nc.dram_tensor("vc", [HALF, D], BF16, kind="ExternalInput")
    outT = nc.dram_tensor("outT", [D, L], BF16, kind="ExternalOutput")
    den = nc.dram_tensor("den", [1, L], F32, kind="ExternalOutput")

    def at(name, shape, dtype, offset):
        return nc.alloc_sbuf_tensor_at(name, shape, dtype, offset=BASE + offset)

    sb = {
        "kT": at("kT", [128, NT, 2 * HALF], BF16, 0),
        "v": at("v", [128, NT, D], BF16, 65536),
        "qt0": at("qt0", [128, NT, 512], BF16, 131072),
        "qt1": at("qt1", [128, NT, 512], BF16, 147456),
        "wva": at("wva", [128, NT, 512], BF16, 131072),
        "wvb": at("wvb", [128, NT, 512], BF16, 147456),
        "x0": at("x0", [128, NT, 512], BF16, 163840),
        "x1": at("x1", [128, NT, 512], BF16, 180224),
        "pt0": at("pt0", [128, NT, 512], BF16, 163840),
        "pt1": at("pt1", [128, NT, 512], BF16, 180224),
        "w0": at("w0", [128, NT, 128], BF16, 196608),
        "w1": at("w1", [128, NT, 128], BF16, 200704),
        "o0": at("o0", [128, 512], BF16, 204800),
        "o1": at("o1", [128, 512], BF16, 205824),
        "o2": at("o2", [128, 512], BF16, 206848),
        "o3": at("o3", [128, 512], BF16, 207872),
        "d0": at("d0", [1, 512], F32, 208896),
        "ones_f": at("ones_f", [128, 1], F32, 210944),
        "ones": at("ones", [128, 1], BF16, 210976),
    }

    from contextlib import ExitStack
    with tile.TileContext(nc) as tc:
        # ones init outside the rep loop (read-only afterwards)
        nc.gpsimd.memset(sb["ones_f"][:], 1.0)
        nc.vector.tensor_copy(sb["ones"][:], sb["ones_f"][:])
        with ExitStack() as _rep_stack:
            if reps > 1:
                _rep_stack.enter_context(
                    tc.For_i(0, reps, 1, hint_engines=tuple(mybir.EngineType))
                )
            _emit_body(nc, tc, sb, xT, wq, wk, wvT, kcT, vc, outT, den)
    nc.compile()
    _NC_CACHE[key] = nc
    return nc


def _emit_body(nc, tc, sb, xT, wq, wk, wvT, kcT, vc, outT, den):
    xT_r = xT.rearrange("(t p) l -> p t l", p=128)
    wvT_r = wvT.rearrange("(t p) d -> p t d", p=128)
    kcT_r = kcT.rearrange("(t p) s -> p t s", p=128)
    vc_r = vc.rearrange("(t p) d -> p t d", p=128)

    kT, v = sb["kT"], sb["v"]
    qt = [sb["qt0"], sb["qt1"]]
    wv = [sb["wva"], sb["wvb"]]
    xc = [sb["x0"], sb["x1"]]
    pt = [sb["pt0"], sb["pt1"]]
    wt = [sb["w0"], sb["w1"]]
    ot = [sb["o0"], sb["o1"], sb["o2"], sb["o3"]]
    dt_sb = [sb["d0"], sb["d0"]]
    ones = sb["ones"]

    with (
        tc.tile_pool(name="psM", bufs=7, space="PSUM") as psM,
        tc.tile_pool(name="psD", bufs=1, space="PSUM") as psDp,
    ):
        # ---- input loads ----
        nc.gpsimd.dma_start(xc[0][:], xT_r[:, :, 0:512])
        nc.scalar.dma_start(wv[0][:], wvT_r[:, :, 0:512])
        nc.gpsimd.dma_start(xc[1][:], xT_r[:, :, 512:1024])
        nc.gpsimd.dma_start(kT[:, :, 0:HALF], kcT_r[:, :, :])
        nc.gpsimd.dma_start(v[:, 0:8, :], vc_r[:, :, :])

        # ---- V projection: V_new[s, d]; lhsT = x cols (stationary) ----
        for dc in range(4):
            wv_sb = wv[dc % 2]
            if dc > 0:
                nc.scalar.dma_start(wv_sb[:],
                                    wvT_r[:, :, dc * 512:(dc + 1) * 512])
            for st8 in range(8):
                sc, so = divmod(st8, 4)
                ps = psM.tile([128, 512], F32, tag="ps")
                for dti in range(NT):
                    nc.tensor.matmul(
                        ps[:], xc[sc][:, dti, so * 128:(so + 1) * 128],
                        wv_sb[:, dti, :],
                        start=(dti == 0), stop=(dti == NT - 1),
                    )
                nc.vector.tensor_copy(
                    v[:, 8 + st8, dc * 512:(dc + 1) * 512], ps[:])

        # ---- K projection: K_new^T[e, s] ----
        for et in range(NT):
            w_sb = wt[et % 2]
            nc.sync.dma_start(w_sb[:], wk[:, et, :, :])
            for sc in range(2):
                ps = psM.tile([128, 512], F32, tag="ps")
                for dti in range(NT):
                    nc.tensor.matmul(
                        ps[:], w_sb[:, dti, :], xc[sc][:, dti, :],
                        start=(dti == 0), stop=(dti == NT - 1),
                    )
                nc.vector.tensor_copy(
                    kT[:, et, HALF + sc * 512:HALF + (sc + 1) * 512], ps[:])

        def q_proj(xs, skip_first_dma=False):
            """Q^T[e, l-half] into qt blocks from x chunk tensors xs."""
            for et in range(NT):
                w_sb = wt[et % 2]
                if not (skip_first_dma and et == 0):
                    nc.sync.dma_start(w_sb[:], wq[:, et, :, :])
                for lc in range(2):
                    ps = psM.tile([128, 512], F32, tag="ps")
                    for dti in range(NT):
                        nc.tensor.matmul(
                            ps[:], w_sb[:, dti, :], xs[lc][:, dti, :],
                            start=(dti == 0), stop=(dti == NT - 1),
                        )
                    nc.vector.tensor_copy(qt[lc][:, et, :], ps[:])

        def attn_block(lbc, gl, prefetch=None):
            """One 512-query block: scores -> exp -> numerator -> den.
            gl = global column offset into outT/den. prefetch: callable
            emitted between the scores and numerator phases."""
            pT = pt[lbc % 2]
            qb = qt[lbc % 2]
            for st in range(NT):
                ps = psM.tile([128, 512], F32, tag="ps")
                for et in range(NT):
                    nc.tensor.matmul(
                        ps[:],
                        kT[:, et, st * 128:(st + 1) * 128],
                        qb[:, et, :],
                        start=(et == 0), stop=(et == NT - 1),
                    )
                nc.scalar.activation(
                    pT[:, st, :], ps[:],
                    mybir.ActivationFunctionType.Exp, scale=SCALE,
                )
            if prefetch is not None:
                prefetch()
            for dti in range(NT):
                ps_o = psM.tile([128, 512], F32, tag="ps")
                for st in range(NT):
                    nc.tensor.matmul(
                        ps_o[:],
                        v[:, st, dti * 128:(dti + 1) * 128],
                        pT[:, st, :],
                        start=(st == 0), stop=(st == NT - 1),
                    )
                o_sb = ot[dti % 4]
                nc.vector.tensor_copy(o_sb[:], ps_o[:])
                nc.sync.dma_start(
                    outT[dti * 128:(dti + 1) * 128, gl:gl + 512], o_sb[:])
            ps_d = psDp.tile([1, 512], F32, tag="psD")
            for st in range(NT):
                nc.tensor.matmul(
                    ps_d[:], ones[:], pT[:, st, :],
                    start=(st == 0), stop=(st == NT - 1),
                )
            d_sb = dt_sb[lbc % 2]
            nc.vector.tensor_copy(d_sb[:], ps_d[:])
            nc.sync.dma_start(den[0:1, gl:gl + 512], d_sb[:])

        # ---- Q projection lh=0 (reuses the kv-half x chunks) ----
        q_proj(xc)

        # ---- attention lh=0; prefetch lh=1 x chunks.
        # x0's region is pT(block0), so its refill waits for block 0 to
        # drain; x1's region is pT(block1), refilled at Q1-proj time.
        attn_block(0, 0)

        def prefetch_x0():
            nc.gpsimd.dma_start(xc[0][:], xT_r[:, :, HALF:HALF + 512])
            nc.scalar.dma_start(wt[0][:], wq[:, 0, :, :])
        attn_block(1, 512, prefetch=prefetch_x0)

        # ---- Q projection lh=1 ----
        nc.gpsimd.dma_start(xc[1][:], xT_r[:, :, HALF + 512:L])
        q_proj(xc, skip_first_dma=True)

        # ---- attention lh=1 ----
        attn_block(0, HALF)
        attn_block(1, HALF + 512)


def _pack_w(wT):
    """[d, e] -> [p, et, dt, 128] with d = dt*128+p, e = et*128+e_lo."""
    return np.ascontiguousarray(
        wT.reshape(NT, 128, NT, 128).transpose(1, 2, 0, 3))


def make_in_maps(x, cache_k, cache_v, Wq, Wk, Wv):
    """Per-core inputs. Core c = (b, h), b = c // 2, h = c % 2. The x
    columns are permuted so the kv-half comes first."""
    f32 = np.float32
    wq_p = _pack_w(np.asarray(Wq, f32).T.astype(BF16_NP))
    wk_p = _pack_w(np.asarray(Wk, f32).T.astype(BF16_NP))
    wvT = np.ascontiguousarray(np.asarray(Wv, f32).T).astype(BF16_NP)
    in_maps = []
    for c in range(N_CORES):
        b, h = divmod(c, 2)
        xb = np.asarray(x[b], f32)
        sl = slice(h * HALF, (h + 1) * HALF)
        ot = slice((1 - h) * HALF, (2 - h) * HALF)
        x_perm = np.concatenate([xb[sl], xb[ot]], axis=0)  # [L, D], kv half first
        in_maps.append({
            "xT": np.ascontiguousarray(x_perm.T).astype(BF16_NP),
            "wq": wq_p,
            "wk": wk_p,
            "wvT": wvT,
            "kcT": np.ascontiguousarray(
                np.asarray(cache_k[b, sl], f32).T).astype(BF16_NP),
            "vc": np.asarray(cache_v[b, sl], f32).astype(BF16_NP),
        })
    return in_maps


def combine(results):
    """out[b] = ((num_h0 + num_h1) / (den_h0 + den_h1)).T, undoing the
    per-core query permutation (core (b,h) processed tokens
    [h*HALF:(h+1)*HALF] first)."""
    B = N_CORES // 2
    out = np.empty((B, L, D), np.float32)
    num = np.empty((D, L), np.float64)
    dent = np.empty(L, np.float64)
    for b in range(B):
        r0, r1 = results[2 * b], results[2 * b + 1]
        # core (b,0): queries in natural order; core (b,1): halves swapped
        n0 = np.asarray(r0["outT"], np.float64)
        n1 = np.asarray(r1["outT"], np.float64)
        d0 = np.asarray(r0["den"][0], np.float64)
        d1 = np.asarray(r1["den"][0], np.float64)
        num[:, 0:HALF] = n0[:, 0:HALF] + n1[:, HALF:L]
        num[:, HALF:L] = n0[:, HALF:L] + n1[:, 0:HALF]
        dent[0:HALF] = d0[0:HALF] + d1[HALF:L]
        dent[HALF:L] = d0[HALF:L] + d1[0:HALF]
        out[b] = (num / dent[None, :]).T.astype(np.float32)
    return out


def kernel(x, cache_k, cache_v, Wq, Wk, Wv):
    nc = build_program()
    in_maps = make_in_maps(x, cache_k, cache_v, Wq, Wk, Wv)
    results = bass2jax.run_bass_via_pjrt(nc, in_maps, n_cores=N_CORES)
    return combine(results)
